# revision 20
# baseline (speedup 1.0000x reference)
"""Multi-head attention (B=2, S=2048, D=1024, H=16) on 8 Trainium2 cores.

Sharding: 2 heads per core (tensor-parallel on H). Each core computes its
2 heads' QKV projections, attention, and a partial output projection
(the 128 columns of the concat dim it owns); the host sums the 8 partial
outputs and adds the output bias.

v2 changes vs baseline:
  - projections run natively in bf16 (no f32r casts of the 25MB x stream)
  - v bias folded to host: softmax rows sum to 1, so the v-bias term is a
    constant row vector bv_concat @ Wo^T added to the final bias on host
  - exp output, v tiles, and ypart are bf16 (halves DVE copy + DMA-out cost)
  - PSUM rebudgeted: scores 2x[128,1024] | o-accum 2x[128,512] |
    misc (proj/transpose/bc/yout) 2x[128,512]
  - q/k bias-adds moved to gpsimd; y-copies alternate DVE/gpsimd

Device dataflow per (batch, head):
  qT/kT = Wq/Wk x^T              [64, S]  bf16 matmuls, psum fp32, +bias
  vT    = Wv xv^T -> PE-transpose -> v_aug natural [S, 64|1] bf16
  sT    = kT^T q                 [t 128, s 512]x2 heads row-packed (psum)
  expS  = exp(sT/8)              (ScalarE, psum -> sbuf bf16)
  o~T/Z = [v|1]^T expS           [65, s] accumulated over t (psum)
  oT    = o~T * (1/Z)            (recip + K=2 PE broadcast + DVE mul)
  y_c   = oT^T Wo_c^T            [s 128, 1024] partial out (psum->bf16->DRAM)
"""

import os
import numpy as np
import ml_dtypes

B, S, D, H = 2, 2048, 1024, 16
HD = D // H          # 64
NCORES = 8
HPC = H // NCORES    # 2 heads per core
P = 128
SC = 512             # s-chunk width
NSC = S // SC        # 4
NKB = D // P         # 8 contraction blocks for projections
NTB = S // P         # 16 t-blocks

_BF16 = ml_dtypes.bfloat16

_nc_cache = {}
_runner_cache = {}


def build_nc(loop_k: int = 1, ablate: str = ""):
    """Build (and cache) the per-core Bass module. loop_k>1 wraps the body
    in a hardware loop for timing measurements. `ablate` is a comma-joined
    set of stage-skip flags for timing experiments (breaks correctness)."""
    key = (loop_k, ablate)
    if key in _nc_cache:
        return _nc_cache[key]
    abl = set(ablate.split(",")) if ablate else set()

    import concourse.bass as bass
    import concourse.mybir as mybir
    import concourse.tile as tile
    from concourse import bacc
    from concourse.masks import make_identity
    from contextlib import ExitStack

    f32 = mybir.dt.float32
    f32r = mybir.dt.float32r
    bf16 = mybir.dt.bfloat16
    AF = mybir.ActivationFunctionType

    nc = bacc.Bacc("TRN2", target_bir_lowering=False)

    xqT = nc.dram_tensor("xqT", [B, D, S], bf16, kind="ExternalInput")
    xkT = nc.dram_tensor("xkT", [B, D, S], bf16, kind="ExternalInput")
    xvT = nc.dram_tensor("xvT", [B, D, S], bf16, kind="ExternalInput")
    wq = nc.dram_tensor("wq", [D, P], bf16, kind="ExternalInput")
    wk = nc.dram_tensor("wk", [D, P], bf16, kind="ExternalInput")
    wv = nc.dram_tensor("wv", [D, P], bf16, kind="ExternalInput")
    bq = nc.dram_tensor("bq", [P, 1], f32, kind="ExternalInput")
    bk = nc.dram_tensor("bk", [P, 1], f32, kind="ExternalInput")
    wo = nc.dram_tensor("wo", [P, D], f32, kind="ExternalInput")
    ypart = nc.dram_tensor("ypart", [B, S, D], bf16, kind="ExternalOutput")

    with tile.TileContext(nc) as tc:
        with ExitStack() as ctx:
            const = ctx.enter_context(tc.tile_pool(name="const", bufs=1))
            xin = ctx.enter_context(tc.tile_pool(name="xin", bufs=24))
            qkv = ctx.enter_context(tc.tile_pool(name="qkv", bufs=2))
            vtp = ctx.enter_context(tc.tile_pool(name="vtp", bufs=2))
            vap = ctx.enter_context(tc.tile_pool(name="vap", bufs=2))
            otp = ctx.enter_context(tc.tile_pool(name="otp", bufs=3))
            expp = ctx.enter_context(tc.tile_pool(name="expp", bufs=8))
            smalls = ctx.enter_context(tc.tile_pool(name="smalls", bufs=4))
            yout = ctx.enter_context(tc.tile_pool(name="yout", bufs=3))
            # PSUM (8 banks): scores 2x[128,1024]=4 | oacc 2x[128,512]=2 |
            # misc (proj chunks, v transposes, bc, yout) 2x[128,512]=2
            pp = ctx.enter_context(tc.tile_pool(name="pp", bufs=2, space="PSUM"))
            ppo = ctx.enter_context(tc.tile_pool(name="ppo", bufs=2, space="PSUM"))
            ppm = ctx.enter_context(tc.tile_pool(name="ppm", bufs=2, space="PSUM"))

            # ---- constants (outside the timing loop) ----
            wq_sb = const.tile([P, NKB, P], bf16, tag="wq")
            wk_sb = const.tile([P, NKB, P], bf16, tag="wk")
            wv_sb = const.tile([P, NKB, P], bf16, tag="wv")
            nc.sync.dma_start(wq_sb[:], wq.ap().rearrange("(a p) e -> p a e", p=P))
            nc.sync.dma_start(wk_sb[:], wk.ap().rearrange("(a p) e -> p a e", p=P))
            nc.sync.dma_start(wv_sb[:], wv.ap().rearrange("(a p) e -> p a e", p=P))
            wo_f32 = const.tile([P, D], f32, tag="wof")
            nc.sync.dma_start(wo_f32[:], wo[:, :])
            wo_sb = const.tile([P, D], bf16, tag="wo")
            nc.vector.tensor_copy(wo_sb[:], wo_f32[:])
            bq_sb = const.tile([P, 1], f32, tag="bq")
            bk_sb = const.tile([P, 1], f32, tag="bk")
            nc.sync.dma_start(bq_sb[:], bq[:, :])
            nc.sync.dma_start(bk_sb[:], bk[:, :])
            ident_f32 = const.tile([P, P], f32, tag="identf")
            make_identity(nc, ident_f32[:])
            ident = const.tile([P, P], bf16, tag="ident")
            nc.vector.tensor_copy(ident[:], ident_f32[:])
            ones_f32 = const.tile([P, HPC * NTB], f32, tag="onesf")
            nc.vector.memset(ones_f32[:], 1.0)
            m_f32 = const.tile([1, 2, P], f32, tag="mf")
            nc.vector.memset(m_f32[:], 0.0)
            nc.vector.memset(m_f32[0:1, 0, 0:HD], 1.0)
            nc.vector.memset(m_f32[0:1, 1, HD:P], 1.0)
            msel = const.tile([1, 2, P], f32r, tag="msel")
            nc.vector.tensor_copy(msel[:], m_f32[:])

            def body():
                # Background work queue: generators yielding after ~one
                # matmul-worth of PE work.  Pumped round-robin inside the
                # (ACT-bound) attention loop so projections for the next
                # batch and output projections fill PE slack.
                bg = []

                def pump(n):
                    for _ in range(n):
                        while bg:
                            try:
                                next(bg[0])
                                bg.append(bg.pop(0))
                                break
                            except StopIteration:
                                bg.pop(0)

                def drain():
                    while bg:
                        pump(1)

                def proj_task(b, tensors, v_aug):
                    """QKV projections for batch b; the v chunks are
                    transposed into v_aug as soon as they are produced.
                    tensors[i] = (w_sb, xdram, b_sb or None, dest)."""
                    nc.vector.tensor_copy(
                        v_aug[:, :, :, HD], ones_f32[:, 0:HPC * NTB]
                    )
                    for (w_sb, xdram, b_sb, dest) in tensors:
                        for half in range(2):
                            hsl = slice(half * (S // 2), (half + 1) * (S // 2))
                            xts = []
                            for kb in range(NKB):
                                xt = xin.tile([P, S // 2], bf16, tag="xt")
                                if "noxdma" not in abl:
                                    nc.sync.dma_start(
                                        xt[:], xdram[b, kb * P:(kb + 1) * P, hsl]
                                    )
                                else:
                                    nc.sync.dma_start(
                                        xt[:, 0:1],
                                        xdram[b, kb * P:(kb + 1) * P,
                                              hsl.start:hsl.start + 1],
                                    )
                                xts.append(xt)
                            yield
                            for sc2 in range(2):
                                sc = half * 2 + sc2
                                ps = ppm.tile([P, SC], f32, tag="m")
                                for kb in range(NKB):
                                    nc.tensor.matmul(
                                        ps[:], w_sb[:, kb, :],
                                        xts[kb][:, sc2 * SC:(sc2 + 1) * SC],
                                        start=(kb == 0), stop=(kb == NKB - 1),
                                    )
                                    if kb % 2 == 1:
                                        yield
                                dsl = dest[:, sc * SC:(sc + 1) * SC]
                                if "noprojdrain" not in abl:
                                    if b_sb is None:
                                        nc.scalar.copy(dsl, ps[:])
                                    else:
                                        nc.scalar.activation(
                                            dsl, ps[:], AF.Identity,
                                            bias=b_sb[:])
                                else:
                                    nc.vector.tensor_copy(
                                        dsl[:, 0:1], ps[:, 0:1])
                                yield
                                if b_sb is None:
                                    # v chunk ready: transpose its 4 t-blocks
                                    for tb in range(4 * sc, 4 * sc + 4):
                                        pst = ppm.tile([P, P], bf16, tag="m")
                                        nc.tensor.transpose(
                                            pst[:],
                                            dest[:, tb * P:(tb + 1) * P],
                                            ident[:],
                                        )
                                        nc.vector.tensor_copy(
                                            v_aug[:, 0:2, tb, 0:HD], pst[:, :]
                                        )
                                        yield

                def yout_task(b, sc, oT_sb):
                    for sb in range(4 * sc, 4 * sc + 4):
                        sbl = sb - 4 * sc
                        osl = oT_sb[:, sbl * P:(sbl + 1) * P]
                        ysb = yout.tile([P, D], bf16, tag="y")
                        for half in range(2):
                            psy = ppm.tile([P, SC], f32, tag="m")
                            nc.tensor.matmul(
                                psy[:], osl, wo_sb[:, half * SC:(half + 1) * SC],
                                start=True, stop=True,
                            )
                            if "noycopy" not in abl:
                                eng = nc.scalar if half == 0 else nc.vector
                                if eng is nc.scalar:
                                    nc.scalar.copy(
                                        ysb[:, half * SC:(half + 1) * SC],
                                        psy[:])
                                else:
                                    nc.vector.tensor_copy(
                                        ysb[:, half * SC:(half + 1) * SC],
                                        psy[:])
                            else:
                                nc.vector.tensor_copy(
                                    ysb[:, half * SC:half * SC + 1],
                                    psy[:, 0:1])
                            yield
                        if "noydma" not in abl:
                            nc.sync.dma_start(
                                ypart[b, sb * P:(sb + 1) * P, :], ysb[:]
                            )

                def batch_tiles(b):
                    qT_sb = qkv.tile([P, S], bf16, tag="qT", name=f"qT{b}")
                    kT_sb = qkv.tile([P, S], bf16, tag="kT", name=f"kT{b}")
                    vT_sb = vtp.tile([P, S], bf16, tag="vT", name=f"vT{b}")
                    v_aug = vap.tile(
                        [P, HPC, NTB, HD + 1], bf16, tag="vaug", name=f"vaug{b}"
                    )
                    tensors = (
                        (wk_sb, xkT, bk_sb, kT_sb),
                        (wq_sb, xqT, bq_sb, qT_sb),
                        (wv_sb, xvT, None, vT_sb),
                    )
                    return qT_sb, kT_sb, vT_sb, v_aug, tensors

                tiles = {0: batch_tiles(0)}

                # prologue: batch 0 projections run unoverlapped
                bg.append(proj_task(0, tiles[0][4], tiles[0][3]))
                drain()

                for b in range(B):
                    # all of batch b's projection work must be emitted
                    # before its attention reads it (emission order defines
                    # the dependency graph)
                    drain()
                    qT_sb, kT_sb, vT_sb, v_aug, _ = tiles[b]
                    for sc in range(NSC):
                        if b + 1 < B and sc == 0:
                            tiles[b + 1] = batch_tiles(b + 1)
                            bg.append(
                                proj_task(b + 1, tiles[b + 1][4],
                                          tiles[b + 1][3])
                            )
                        oT_sb = otp.tile([P, SC], bf16, tag="oT")
                        o_h0 = ppo.tile([HD + 1, SC], f32, tag="oacc")
                        o_h1 = ppo.tile([HD + 1, SC], f32, tag="oacc")
                        for tb in range(NTB):
                            ssl = slice(sc * SC, (sc + 1) * SC)
                            tsl = slice(tb * P, (tb + 1) * P)
                            ps_sc = pp.tile([P, 2 * SC], f32, tag="sc")
                            nc.tensor.matmul(
                                ps_sc[:, 0:SC], kT_sb[0:HD, tsl], qT_sb[0:HD, ssl],
                                start=True, stop=True, tile_position=(0, 0),
                            )
                            nc.tensor.matmul(
                                ps_sc[:, SC:2 * SC], kT_sb[HD:P, tsl], qT_sb[HD:P, ssl],
                                start=True, stop=True, tile_position=(64, 0),
                            )
                            ex = expp.tile([P, 2 * SC], bf16, tag="ex")
                            if "noexp" not in abl:
                                nc.scalar.activation(ex[:], ps_sc[:], AF.Exp, scale=0.125)
                            else:
                                nc.scalar.activation(
                                    ex[:, 0:8], ps_sc[:, 0:8], AF.Exp, scale=0.125)
                            nc.tensor.matmul(
                                o_h0[:], v_aug[:, 0, tb, :], ex[:, 0:SC],
                                start=(tb == 0), stop=(tb == NTB - 1),
                            )
                            nc.tensor.matmul(
                                o_h1[:], v_aug[:, 1, tb, :], ex[:, SC:2 * SC],
                                start=(tb == 0), stop=(tb == NTB - 1),
                            )
                            pump(2)
                        # normalize both heads for this s-chunk
                        if "nonorm" in abl:
                            nc.vector.tensor_copy(
                                oT_sb[0:HD, 0:1], o_h0[0:HD, 0:1])
                            nc.vector.tensor_copy(
                                oT_sb[HD:P, 0:1], o_h1[0:HD, 0:1])
                        else:
                            rzdt = f32r if "pebc" in abl else f32
                            rz0 = smalls.tile([1, SC], rzdt, tag="rz")
                            rz1 = smalls.tile([1, SC], rzdt, tag="rz")
                            with nc.allow_low_precision(
                                reason="fp32 recip feeding broadcast"
                            ):
                                if "norecip" in abl:
                                    nc.vector.tensor_copy(
                                        rz0[:], o_h0[HD:HD + 1, :])
                                    nc.vector.tensor_copy(
                                        rz1[:], o_h1[HD:HD + 1, :])
                                else:
                                    nc.vector.reciprocal(
                                        rz0[:], o_h0[HD:HD + 1, :])
                                    nc.vector.reciprocal(
                                        rz1[:], o_h1[HD:HD + 1, :])
                            if "pebc" in abl:
                                bcp = ppm.tile([P, SC], f32, tag="m")
                                nc.tensor.matmul(
                                    bcp[:], msel[0:1, 0, :], rz0[:],
                                    start=True, stop=False)
                                nc.tensor.matmul(
                                    bcp[:], msel[0:1, 1, :], rz1[:],
                                    start=False, stop=True)
                                bcs = smalls.tile([P, SC], f32, tag="bc0")
                                nc.scalar.copy(bcs[:], bcp[:])
                                nc.vector.tensor_mul(
                                    oT_sb[0:HD, :], o_h0[0:HD, :], bcs[0:HD, :])
                                nc.vector.tensor_mul(
                                    oT_sb[HD:P, :], o_h1[0:HD, :], bcs[HD:P, :])
                            else:
                                bc0 = smalls.tile([P, SC], f32, tag="bc0")
                                bc1 = smalls.tile([P, SC], f32, tag="bc1")
                                if "nobcast" in abl:
                                    nc.vector.memset(bc0[:, 0:1], 1.0)
                                    nc.vector.memset(bc1[:, 0:1], 1.0)
                                else:
                                    nc.gpsimd.partition_broadcast(bc0[:], rz0[:])
                                    nc.gpsimd.partition_broadcast(bc1[:], rz1[:])
                                nc.vector.tensor_mul(
                                    oT_sb[0:HD, :], o_h0[0:HD, :], bc0[0:HD, :]
                                )
                                nc.vector.tensor_mul(
                                    oT_sb[HD:P, :], o_h1[0:HD, :], bc1[0:HD, :]
                                )
                        bg.append(yout_task(b, sc, oT_sb))
                drain()

            if loop_k == 1:
                body()
            else:
                with tc.For_i(
                    0, loop_k, 1,
                    hint_engines=(
                        mybir.EngineType.PE,
                        mybir.EngineType.DVE,
                        mybir.EngineType.Activation,
                        mybir.EngineType.SP,
                        mybir.EngineType.Pool,
                    ),
                ):
                    body()

    nc.compile()
    _nc_cache[key] = nc
    return nc


def make_in_maps(inputs):
    """Host-side sharding: transpose activations to [B, D, S] bf16, slice
    per-head weights per core."""
    query, key, value = inputs["query"], inputs["key"], inputs["value"]
    Wq, bq, Wk, bk, Wv = (
        inputs["Wq"], inputs["bq"], inputs["Wk"], inputs["bk"], inputs["Wv"],
    )
    Wo = inputs["Wo"]

    xqT = np.ascontiguousarray(np.transpose(query, (0, 2, 1))).astype(_BF16)
    xkT = np.ascontiguousarray(np.transpose(key, (0, 2, 1))).astype(_BF16)
    xvT = np.ascontiguousarray(np.transpose(value, (0, 2, 1))).astype(_BF16)

    in_maps = []
    for c in range(NCORES):
        hs = slice(c * HPC, (c + 1) * HPC)
        # [HPC, HD, D] -> [D, HPC*HD]
        wq_c = np.ascontiguousarray(
            Wq[hs].reshape(HPC * HD, D).T).astype(_BF16)
        wk_c = np.ascontiguousarray(
            Wk[hs].reshape(HPC * HD, D).T).astype(_BF16)
        wv_c = np.ascontiguousarray(
            Wv[hs].reshape(HPC * HD, D).T).astype(_BF16)
        bq_c = np.ascontiguousarray(bq[hs].reshape(P, 1)).astype(np.float32)
        bk_c = np.ascontiguousarray(bk[hs].reshape(P, 1)).astype(np.float32)
        wo_c = np.ascontiguousarray(Wo[:, c * P:(c + 1) * P].T).astype(np.float32)
        in_maps.append({
            "xqT": xqT, "xkT": xkT, "xvT": xvT,
            "wq": wq_c, "wk": wk_c, "wv": wv_c,
            "bq": bq_c, "bk": bk_c,
            "wo": wo_c,
        })
    return in_maps


def make_runner(nc, n_cores=NCORES):
    """Cached jitted shard_map runner (mirrors bass2jax.run_bass_via_pjrt
    without donation so it can be re-invoked for timing)."""
    key = id(nc)
    if key in _runner_cache:
        return _runner_cache[key]
    import jax
    from jax.sharding import Mesh, PartitionSpec
    from jax.experimental.shard_map import shard_map
    import concourse.mybir as mybir
    from concourse import bass2jax

    bass2jax.install_neuronx_cc_hook()
    partition_name = nc.partition_id_tensor.name if nc.partition_id_tensor else None
    in_names, out_names, out_avals = [], [], []
    for alloc in nc.m.functions[0].allocations:
        if not isinstance(alloc, mybir.MemoryLocationSet):
            continue
        name = alloc.memorylocations[0].name
        if alloc.kind == "ExternalInput":
            if name != partition_name:
                in_names.append(name)
        elif alloc.kind == "ExternalOutput":
            out_names.append(name)
            out_avals.append(
                jax.core.ShapedArray(
                    tuple(alloc.tensor_shape), mybir.dt.np(alloc.dtype))
            )
    all_in_names = list(in_names) + ([partition_name] if partition_name else [])

    def _body(*args):
        operands = list(args)
        if partition_name is not None:
            operands.append(bass2jax.partition_id_tensor())
        outs = bass2jax._bass_exec_p.bind(
            *operands, out_avals=tuple(out_avals),
            in_names=tuple(all_in_names), out_names=tuple(out_names),
            lowering_input_output_aliases=(),
            sim_require_finite=False, sim_require_nnan=False, nc=nc)
        return tuple(outs)

    devices = jax.devices()[:n_cores]
    mesh = Mesh(np.asarray(devices), ("core",))
    fn = jax.jit(shard_map(
        _body, mesh=mesh,
        in_specs=(PartitionSpec("core"),) * len(in_names),
        out_specs=(PartitionSpec("core"),) * len(out_names),
        check_rep=False))
    out = (fn, in_names, out_names, out_avals)
    _runner_cache[key] = out
    return out


def run_on_cores(nc, in_maps):
    """Run the module on the 8 cores; returns list of per-core out dicts."""
    import jax
    fn, in_names, out_names, out_avals = make_runner(nc)
    concat_in = [
        np.concatenate([m[nm] for m in in_maps], axis=0) for nm in in_names
    ]
    outs = jax.block_until_ready(fn(*concat_in))
    res = []
    for c in range(len(in_maps)):
        d = {}
        for i, nm in enumerate(out_names):
            shp = out_avals[i].shape
            d[nm] = np.asarray(outs[i]).reshape(len(in_maps), *shp)[c]
        res.append(d)
    return res


def postprocess(results, inputs):
    """Sum per-core partial outputs; add output bias and the host-folded
    v-bias term (softmax rows sum to 1 => + bv_concat @ Wo^T exactly)."""
    acc = np.zeros((B, S, D), dtype=np.float64)
    for r in results:
        acc += r["ypart"].astype(np.float64)
    bv_concat = inputs["bv"].astype(np.float64).reshape(D)
    acc += inputs["bo"].astype(np.float64)
    acc += bv_concat @ inputs["Wo"].astype(np.float64).T
    return acc.astype(np.float32)


def kernel(**inputs) -> np.ndarray:
    inputs = {k: np.asarray(v) for k, v in inputs.items()}
    nc = build_nc(loop_k=1)
    in_maps = make_in_maps(inputs)
    results = run_on_cores(nc, in_maps)
    return postprocess(results, inputs)


# revision 22
# speedup vs baseline: 1.0385x; 1.0385x over previous
"""Multi-head attention (B=2, S=2048, D=1024, H=16) on 8 Trainium2 cores.

Sharding: 2 heads per core (tensor-parallel on H). Each core computes its
2 heads' QKV projections, attention, and a partial output projection
(the 128 columns of the concat dim it owns); the host sums the 8 partial
outputs and adds the output bias (plus the host-folded v-bias term:
softmax rows sum to 1, so bv contributes the constant bv_concat @ Wo^T).

Key design points (arrived at via HW ablation, see session notes):
  - all matmuls bf16 (x stream loads straight from DRAM, no casts)
  - exp on ScalarE from the score PSUM tiles, bf16 out -> P@V rhs
  - PSUM->SBUF drains are the scarce resource (DVE fp32-psum runs 1x with
    a full pipeline DRAIN per op; ScalarE is (172+FD)/1.2GHz): drains are
    split across ScalarE (proj bias-adds via activation bias, v chunks,
    half the y tiles) and DVE (rest)
  - 1/Z broadcast via gpsimd partition_broadcast (PE outer-product and
    ScalarE-copy variants measured slower)
  - software pipelining: a generator-based background queue interleaves
    next-batch projections + v transposes + output projections into the
    attention tb loop (pump(2) per tile) so PE slack under the ACT-bound
    exp stream is filled
  - PSUM (8 banks): scores 2x[128,1024] | o-accum 2x[65,512] |
    misc ring 2x[128,512]

Device dataflow per (batch, head):
  qT/kT = Wq/Wk x^T + b          [64, S]   bf16, psum fp32, ScE bias-add
  vT    = Wv xv^T -> PE-transpose -> v_aug natural [S, 64|1] bf16
  sT    = kT^T q                 [t 128, s 512]x2 heads row-packed (psum)
  expS  = exp(sT/8)              (ScalarE, psum -> sbuf bf16)
  o~T/Z = [v|1]^T expS           [65, s] accumulated over t (psum)
  oT    = o~T * (1/Z)            (DVE recip + gpsimd bcast + DVE mul)
  y_c   = oT^T Wo_c^T            [s 128, 1024] partial out (psum->bf16->DRAM)

`ablate` flags in build_nc are timing-experiment switches; the default
("") is the correct, fastest configuration.
"""

import os
import numpy as np
import ml_dtypes

B, S, D, H = 2, 2048, 1024, 16
HD = D // H          # 64
NCORES = 8
HPC = H // NCORES    # 2 heads per core
P = 128
SC = 512             # s-chunk width
NSC = S // SC        # 4
NKB = D // P         # 8 contraction blocks for projections
NTB = S // P         # 16 t-blocks

_BF16 = ml_dtypes.bfloat16

_nc_cache = {}
_runner_cache = {}


def build_nc(loop_k: int = 1, ablate: str = ""):
    """Build (and cache) the per-core Bass module. loop_k>1 wraps the body
    in a hardware loop for timing measurements. `ablate` is a comma-joined
    set of stage-skip flags for timing experiments (breaks correctness)."""
    key = (loop_k, ablate)
    if key in _nc_cache:
        return _nc_cache[key]
    abl = set(ablate.split(",")) if ablate else set()

    import concourse.bass as bass
    import concourse.mybir as mybir
    import concourse.tile as tile
    from concourse import bacc
    from concourse.masks import make_identity
    from contextlib import ExitStack

    f32 = mybir.dt.float32
    f32r = mybir.dt.float32r
    bf16 = mybir.dt.bfloat16
    AF = mybir.ActivationFunctionType

    nc = bacc.Bacc("TRN2", target_bir_lowering=False)

    xqT = nc.dram_tensor("xqT", [B, D, S], bf16, kind="ExternalInput")
    xkT = nc.dram_tensor("xkT", [B, D, S], bf16, kind="ExternalInput")
    xvT = nc.dram_tensor("xvT", [B, D, S], bf16, kind="ExternalInput")
    wq = nc.dram_tensor("wq", [D, P], bf16, kind="ExternalInput")
    wk = nc.dram_tensor("wk", [D, P], bf16, kind="ExternalInput")
    wv = nc.dram_tensor("wv", [D, P], bf16, kind="ExternalInput")
    bq = nc.dram_tensor("bq", [P, 1], f32, kind="ExternalInput")
    bk = nc.dram_tensor("bk", [P, 1], f32, kind="ExternalInput")
    wo = nc.dram_tensor("wo", [P, D], f32, kind="ExternalInput")
    ypart = nc.dram_tensor("ypart", [B, S, D], bf16, kind="ExternalOutput")

    with tile.TileContext(nc) as tc:
        with ExitStack() as ctx:
            const = ctx.enter_context(tc.tile_pool(name="const", bufs=1))
            xin = ctx.enter_context(tc.tile_pool(name="xin", bufs=24))
            qkv = ctx.enter_context(tc.tile_pool(name="qkv", bufs=2))
            vtp = ctx.enter_context(tc.tile_pool(name="vtp", bufs=2))
            vap = ctx.enter_context(tc.tile_pool(name="vap", bufs=2))
            otp = ctx.enter_context(tc.tile_pool(name="otp", bufs=3))
            expp = ctx.enter_context(tc.tile_pool(name="expp", bufs=8))
            smalls = ctx.enter_context(tc.tile_pool(name="smalls", bufs=4))
            yout = ctx.enter_context(tc.tile_pool(name="yout", bufs=3))
            # PSUM (8 banks): scores 2x[128,1024]=4 | oacc 2x[128,512]=2 |
            # misc (proj chunks, v transposes, bc, yout) 2x[128,512]=2
            pp = ctx.enter_context(tc.tile_pool(name="pp", bufs=2, space="PSUM"))
            ppo = ctx.enter_context(tc.tile_pool(name="ppo", bufs=2, space="PSUM"))
            ppm = ctx.enter_context(tc.tile_pool(name="ppm", bufs=2, space="PSUM"))

            # ---- constants (outside the timing loop) ----
            wq_sb = const.tile([P, NKB, P], bf16, tag="wq")
            wk_sb = const.tile([P, NKB, P], bf16, tag="wk")
            wv_sb = const.tile([P, NKB, P], bf16, tag="wv")
            nc.sync.dma_start(wq_sb[:], wq.ap().rearrange("(a p) e -> p a e", p=P))
            nc.sync.dma_start(wk_sb[:], wk.ap().rearrange("(a p) e -> p a e", p=P))
            nc.sync.dma_start(wv_sb[:], wv.ap().rearrange("(a p) e -> p a e", p=P))
            wo_f32 = const.tile([P, D], f32, tag="wof")
            nc.sync.dma_start(wo_f32[:], wo[:, :])
            wo_sb = const.tile([P, D], bf16, tag="wo")
            nc.vector.tensor_copy(wo_sb[:], wo_f32[:])
            bq_sb = const.tile([P, 1], f32, tag="bq")
            bk_sb = const.tile([P, 1], f32, tag="bk")
            nc.sync.dma_start(bq_sb[:], bq[:, :])
            nc.sync.dma_start(bk_sb[:], bk[:, :])
            ident_f32 = const.tile([P, P], f32, tag="identf")
            make_identity(nc, ident_f32[:])
            ident = const.tile([P, P], bf16, tag="ident")
            nc.vector.tensor_copy(ident[:], ident_f32[:])
            ones_f32 = const.tile([P, HPC * NTB], f32, tag="onesf")
            nc.vector.memset(ones_f32[:], 1.0)
            m_f32 = const.tile([1, 2, P], f32, tag="mf")
            nc.vector.memset(m_f32[:], 0.0)
            nc.vector.memset(m_f32[0:1, 0, 0:HD], 1.0)
            nc.vector.memset(m_f32[0:1, 1, HD:P], 1.0)
            msel = const.tile([1, 2, P], f32r, tag="msel")
            nc.vector.tensor_copy(msel[:], m_f32[:])

            def body():
                # Background work queue: generators yielding after ~one
                # matmul-worth of PE work.  Pumped round-robin inside the
                # (ACT-bound) attention loop so projections for the next
                # batch and output projections fill PE slack.
                bg = []

                def pump(n):
                    for _ in range(n):
                        while bg:
                            try:
                                next(bg[0])
                                bg.append(bg.pop(0))
                                break
                            except StopIteration:
                                bg.pop(0)

                def drain():
                    while bg:
                        pump(1)

                def proj_task(b, tensors, v_aug):
                    """QKV projections for batch b; the v chunks are
                    transposed into v_aug as soon as they are produced.
                    tensors[i] = (w_sb, xdram, b_sb or None, dest)."""
                    nc.vector.tensor_copy(
                        v_aug[:, :, :, HD], ones_f32[:, 0:HPC * NTB]
                    )
                    for (w_sb, xdram, b_sb, dest) in tensors:
                        for half in range(2):
                            hsl = slice(half * (S // 2), (half + 1) * (S // 2))
                            xts = []
                            for kb in range(NKB):
                                xt = xin.tile([P, S // 2], bf16, tag="xt")
                                if "noxdma" not in abl:
                                    nc.sync.dma_start(
                                        xt[:], xdram[b, kb * P:(kb + 1) * P, hsl]
                                    )
                                else:
                                    nc.sync.dma_start(
                                        xt[:, 0:1],
                                        xdram[b, kb * P:(kb + 1) * P,
                                              hsl.start:hsl.start + 1],
                                    )
                                xts.append(xt)
                            yield
                            for sc2 in range(2):
                                sc = half * 2 + sc2
                                ps = ppm.tile([P, SC], f32, tag="m")
                                for kb in range(NKB):
                                    nc.tensor.matmul(
                                        ps[:], w_sb[:, kb, :],
                                        xts[kb][:, sc2 * SC:(sc2 + 1) * SC],
                                        start=(kb == 0), stop=(kb == NKB - 1),
                                    )
                                    if kb % 2 == 1:
                                        yield
                                dsl = dest[:, sc * SC:(sc + 1) * SC]
                                if "noprojdrain" not in abl:
                                    if b_sb is None:
                                        if "dvev" in abl:
                                            nc.vector.tensor_copy(dsl, ps[:])
                                        else:
                                            nc.scalar.copy(dsl, ps[:])
                                    else:
                                        nc.scalar.activation(
                                            dsl, ps[:], AF.Identity,
                                            bias=b_sb[:])
                                else:
                                    nc.vector.tensor_copy(
                                        dsl[:, 0:1], ps[:, 0:1])
                                yield
                                if b_sb is None:
                                    # v chunk ready: transpose its 4 t-blocks
                                    for tb in range(4 * sc, 4 * sc + 4):
                                        pst = ppm.tile([P, P], bf16, tag="m")
                                        nc.tensor.transpose(
                                            pst[:],
                                            dest[:, tb * P:(tb + 1) * P],
                                            ident[:],
                                        )
                                        nc.vector.tensor_copy(
                                            v_aug[:, 0:2, tb, 0:HD], pst[:, :]
                                        )
                                        yield

                def yout_task(b, sc, oT_sb):
                    for sb in range(4 * sc, 4 * sc + 4):
                        sbl = sb - 4 * sc
                        osl = oT_sb[:, sbl * P:(sbl + 1) * P]
                        ysb = yout.tile([P, D], bf16, tag="y")
                        for half in range(2):
                            psy = ppm.tile([P, SC], f32, tag="m")
                            nc.tensor.matmul(
                                psy[:], osl, wo_sb[:, half * SC:(half + 1) * SC],
                                start=True, stop=True,
                            )
                            if "noycopy" not in abl:
                                eng = (nc.vector if "dvey" in abl
                                       else (nc.scalar if half == 0
                                             else nc.vector))
                                if eng is nc.scalar:
                                    nc.scalar.copy(
                                        ysb[:, half * SC:(half + 1) * SC],
                                        psy[:])
                                else:
                                    nc.vector.tensor_copy(
                                        ysb[:, half * SC:(half + 1) * SC],
                                        psy[:])
                            else:
                                nc.vector.tensor_copy(
                                    ysb[:, half * SC:half * SC + 1],
                                    psy[:, 0:1])
                            yield
                        if "noydma" not in abl:
                            nc.sync.dma_start(
                                ypart[b, sb * P:(sb + 1) * P, :], ysb[:]
                            )

                def batch_tiles(b):
                    qT_sb = qkv.tile([P, S], bf16, tag="qT", name=f"qT{b}")
                    kT_sb = qkv.tile([P, S], bf16, tag="kT", name=f"kT{b}")
                    vT_sb = vtp.tile([P, S], bf16, tag="vT", name=f"vT{b}")
                    v_aug = vap.tile(
                        [P, HPC, NTB, HD + 1], bf16, tag="vaug", name=f"vaug{b}"
                    )
                    tensors = (
                        (wk_sb, xkT, bk_sb, kT_sb),
                        (wq_sb, xqT, bq_sb, qT_sb),
                        (wv_sb, xvT, None, vT_sb),
                    )
                    return qT_sb, kT_sb, vT_sb, v_aug, tensors

                tiles = {0: batch_tiles(0)}

                # prologue: batch 0 projections run unoverlapped
                bg.append(proj_task(0, tiles[0][4], tiles[0][3]))
                drain()

                for b in range(B):
                    # all of batch b's projection work must be emitted
                    # before its attention reads it (emission order defines
                    # the dependency graph)
                    drain()
                    qT_sb, kT_sb, vT_sb, v_aug, _ = tiles[b]
                    for sc in range(NSC):
                        if b + 1 < B and sc == 0:
                            tiles[b + 1] = batch_tiles(b + 1)
                            bg.append(
                                proj_task(b + 1, tiles[b + 1][4],
                                          tiles[b + 1][3])
                            )
                        oT_sb = otp.tile([P, SC], bf16, tag="oT")
                        o_h0 = ppo.tile([HD + 1, SC], f32, tag="oacc")
                        o_h1 = ppo.tile([HD + 1, SC], f32, tag="oacc")
                        for tb in range(NTB):
                            ssl = slice(sc * SC, (sc + 1) * SC)
                            tsl = slice(tb * P, (tb + 1) * P)
                            ps_sc = pp.tile([P, 2 * SC], f32, tag="sc")
                            nc.tensor.matmul(
                                ps_sc[:, 0:SC], kT_sb[0:HD, tsl], qT_sb[0:HD, ssl],
                                start=True, stop=True, tile_position=(0, 0),
                            )
                            nc.tensor.matmul(
                                ps_sc[:, SC:2 * SC], kT_sb[HD:P, tsl], qT_sb[HD:P, ssl],
                                start=True, stop=True, tile_position=(64, 0),
                            )
                            ex = expp.tile([P, 2 * SC], bf16, tag="ex")
                            if "noexp" not in abl:
                                nc.scalar.activation(ex[:], ps_sc[:], AF.Exp, scale=0.125)
                            else:
                                nc.scalar.activation(
                                    ex[:, 0:8], ps_sc[:, 0:8], AF.Exp, scale=0.125)
                            nc.tensor.matmul(
                                o_h0[:], v_aug[:, 0, tb, :], ex[:, 0:SC],
                                start=(tb == 0), stop=(tb == NTB - 1),
                            )
                            nc.tensor.matmul(
                                o_h1[:], v_aug[:, 1, tb, :], ex[:, SC:2 * SC],
                                start=(tb == 0), stop=(tb == NTB - 1),
                            )
                            pump(2)
                        # normalize both heads for this s-chunk
                        if "nonorm" in abl:
                            nc.vector.tensor_copy(
                                oT_sb[0:HD, 0:1], o_h0[0:HD, 0:1])
                            nc.vector.tensor_copy(
                                oT_sb[HD:P, 0:1], o_h1[0:HD, 0:1])
                        else:
                            rzdt = f32r if "pebc" in abl else f32
                            rz0 = smalls.tile([1, SC], rzdt, tag="rz")
                            rz1 = smalls.tile([1, SC], rzdt, tag="rz")
                            with nc.allow_low_precision(
                                reason="fp32 recip feeding broadcast"
                            ):
                                if "norecip" in abl:
                                    nc.vector.tensor_copy(
                                        rz0[:], o_h0[HD:HD + 1, :])
                                    nc.vector.tensor_copy(
                                        rz1[:], o_h1[HD:HD + 1, :])
                                else:
                                    nc.vector.reciprocal(
                                        rz0[:], o_h0[HD:HD + 1, :])
                                    nc.vector.reciprocal(
                                        rz1[:], o_h1[HD:HD + 1, :])
                            if "pebc" in abl:
                                bcp = ppm.tile([P, SC], f32, tag="m")
                                nc.tensor.matmul(
                                    bcp[:], msel[0:1, 0, :], rz0[:],
                                    start=True, stop=False)
                                nc.tensor.matmul(
                                    bcp[:], msel[0:1, 1, :], rz1[:],
                                    start=False, stop=True)
                                bcs = smalls.tile([P, SC], f32, tag="bc0")
                                nc.scalar.copy(bcs[:], bcp[:])
                                nc.vector.tensor_mul(
                                    oT_sb[0:HD, :], o_h0[0:HD, :], bcs[0:HD, :])
                                nc.vector.tensor_mul(
                                    oT_sb[HD:P, :], o_h1[0:HD, :], bcs[HD:P, :])
                            else:
                                bc0 = smalls.tile([P, SC], f32, tag="bc0")
                                bc1 = smalls.tile([P, SC], f32, tag="bc1")
                                if "nobcast" in abl:
                                    nc.vector.memset(bc0[:, 0:1], 1.0)
                                    nc.vector.memset(bc1[:, 0:1], 1.0)
                                else:
                                    nc.gpsimd.partition_broadcast(bc0[:], rz0[:])
                                    nc.gpsimd.partition_broadcast(bc1[:], rz1[:])
                                nc.vector.tensor_mul(
                                    oT_sb[0:HD, :], o_h0[0:HD, :], bc0[0:HD, :]
                                )
                                nc.vector.tensor_mul(
                                    oT_sb[HD:P, :], o_h1[0:HD, :], bc1[0:HD, :]
                                )
                        bg.append(yout_task(b, sc, oT_sb))
                drain()

            if loop_k == 1:
                body()
            else:
                with tc.For_i(
                    0, loop_k, 1,
                    hint_engines=(
                        mybir.EngineType.PE,
                        mybir.EngineType.DVE,
                        mybir.EngineType.Activation,
                        mybir.EngineType.SP,
                        mybir.EngineType.Pool,
                    ),
                ):
                    body()

    nc.compile()
    _nc_cache[key] = nc
    return nc


def make_in_maps(inputs):
    """Host-side sharding: transpose activations to [B, D, S] bf16, slice
    per-head weights per core."""
    query, key, value = inputs["query"], inputs["key"], inputs["value"]
    Wq, bq, Wk, bk, Wv = (
        inputs["Wq"], inputs["bq"], inputs["Wk"], inputs["bk"], inputs["Wv"],
    )
    Wo = inputs["Wo"]

    xqT = np.ascontiguousarray(np.transpose(query, (0, 2, 1))).astype(_BF16)
    xkT = np.ascontiguousarray(np.transpose(key, (0, 2, 1))).astype(_BF16)
    xvT = np.ascontiguousarray(np.transpose(value, (0, 2, 1))).astype(_BF16)

    in_maps = []
    for c in range(NCORES):
        hs = slice(c * HPC, (c + 1) * HPC)
        # [HPC, HD, D] -> [D, HPC*HD]
        wq_c = np.ascontiguousarray(
            Wq[hs].reshape(HPC * HD, D).T).astype(_BF16)
        wk_c = np.ascontiguousarray(
            Wk[hs].reshape(HPC * HD, D).T).astype(_BF16)
        wv_c = np.ascontiguousarray(
            Wv[hs].reshape(HPC * HD, D).T).astype(_BF16)
        bq_c = np.ascontiguousarray(bq[hs].reshape(P, 1)).astype(np.float32)
        bk_c = np.ascontiguousarray(bk[hs].reshape(P, 1)).astype(np.float32)
        wo_c = np.ascontiguousarray(Wo[:, c * P:(c + 1) * P].T).astype(np.float32)
        in_maps.append({
            "xqT": xqT, "xkT": xkT, "xvT": xvT,
            "wq": wq_c, "wk": wk_c, "wv": wv_c,
            "bq": bq_c, "bk": bk_c,
            "wo": wo_c,
        })
    return in_maps


def make_runner(nc, n_cores=NCORES):
    """Cached jitted shard_map runner (mirrors bass2jax.run_bass_via_pjrt
    without donation so it can be re-invoked for timing)."""
    key = id(nc)
    if key in _runner_cache:
        return _runner_cache[key]
    import jax
    from jax.sharding import Mesh, PartitionSpec
    from jax.experimental.shard_map import shard_map
    import concourse.mybir as mybir
    from concourse import bass2jax

    bass2jax.install_neuronx_cc_hook()
    partition_name = nc.partition_id_tensor.name if nc.partition_id_tensor else None
    in_names, out_names, out_avals = [], [], []
    for alloc in nc.m.functions[0].allocations:
        if not isinstance(alloc, mybir.MemoryLocationSet):
            continue
        name = alloc.memorylocations[0].name
        if alloc.kind == "ExternalInput":
            if name != partition_name:
                in_names.append(name)
        elif alloc.kind == "ExternalOutput":
            out_names.append(name)
            out_avals.append(
                jax.core.ShapedArray(
                    tuple(alloc.tensor_shape), mybir.dt.np(alloc.dtype))
            )
    all_in_names = list(in_names) + ([partition_name] if partition_name else [])

    def _body(*args):
        operands = list(args)
        if partition_name is not None:
            operands.append(bass2jax.partition_id_tensor())
        outs = bass2jax._bass_exec_p.bind(
            *operands, out_avals=tuple(out_avals),
            in_names=tuple(all_in_names), out_names=tuple(out_names),
            lowering_input_output_aliases=(),
            sim_require_finite=False, sim_require_nnan=False, nc=nc)
        return tuple(outs)

    devices = jax.devices()[:n_cores]
    mesh = Mesh(np.asarray(devices), ("core",))
    fn = jax.jit(shard_map(
        _body, mesh=mesh,
        in_specs=(PartitionSpec("core"),) * len(in_names),
        out_specs=(PartitionSpec("core"),) * len(out_names),
        check_rep=False))
    out = (fn, in_names, out_names, out_avals)
    _runner_cache[key] = out
    return out


def run_on_cores(nc, in_maps):
    """Run the module on the 8 cores; returns list of per-core out dicts."""
    import jax
    fn, in_names, out_names, out_avals = make_runner(nc)
    concat_in = [
        np.concatenate([m[nm] for m in in_maps], axis=0) for nm in in_names
    ]
    outs = jax.block_until_ready(fn(*concat_in))
    res = []
    for c in range(len(in_maps)):
        d = {}
        for i, nm in enumerate(out_names):
            shp = out_avals[i].shape
            d[nm] = np.asarray(outs[i]).reshape(len(in_maps), *shp)[c]
        res.append(d)
    return res


def postprocess(results, inputs):
    """Sum per-core partial outputs; add output bias and the host-folded
    v-bias term (softmax rows sum to 1 => + bv_concat @ Wo^T exactly)."""
    acc = np.zeros((B, S, D), dtype=np.float64)
    for r in results:
        acc += r["ypart"].astype(np.float64)
    bv_concat = inputs["bv"].astype(np.float64).reshape(D)
    acc += inputs["bo"].astype(np.float64)
    acc += bv_concat @ inputs["Wo"].astype(np.float64).T
    return acc.astype(np.float32)


def kernel(**inputs) -> np.ndarray:
    inputs = {k: np.asarray(v) for k, v in inputs.items()}
    nc = build_nc(loop_k=1)
    in_maps = make_in_maps(inputs)
    results = run_on_cores(nc, in_maps)
    return postprocess(results, inputs)
